# revision 11
# baseline (speedup 1.0000x reference)
"""GAT 2-layer kernel for Trainium2, 8 NeuronCores (Bass/Tile).

Strategy (graph/data parallel per the sharding hint):
  - Nodes are degree-sorted and dealt round-robin to the 8 cores; each core
    owns the edges whose dst it owns, so edge-softmax and the weighted
    aggregation are core-local.
  - Per GAT layer, two SPMD launches:
      A-launch: table build, sharded by node id - core c computes rows of
        T = X @ [W | W@al | W@ar]; h is written as a packed table, el/er as
        a small f32 side table.
      B-launch: batches of dst-node tiles; per batch, gather all in-edge
        source rows with dma_gather, then a fused attention pipeline: one
        add (el+er), leaky-relu+exp on the scalar engine, one broadcast
        multiply, one slot reduce, fold, normalize, bias, activation.
  - The host only routes bytes between launches (shard/gather/concat); all
    arithmetic runs on device.

Table packing: rows hold a PAIR of nodes in 256 bytes (the dma_gather
minimum element), idx = table_row//2, so the table footprint is halved
(6.4MB) and HBM random reads stay fast:
  layer 1: 2 x 128 fp8(e3m4) cols; layer 2: 2 x 64 bf16 cols.
The wrong pair-mate is masked by el = -1e30 => softmax weight 0. Machine
heads m = parity*heads + h (4 for layer 1, 2 for layer 2).
"""

import os
import sys
import types
import numpy as np

sys.path.insert(0, "/opt/trn_rl_repo")

N = 50000
E = 800000
CIN = 128
NCORES = 8
NSH = N // NCORES            # 6250 nodes per core
TB = (NSH + 127) // 128      # 49 dst tiles per core
NSHPAD = TB * 128            # 6272
NPAD = NCORES * NSHPAD       # 50176 table rows
TBAT = 4                     # dst tiles per batch in the B-launch
GCHUNK = 8                   # max slots (1024 idxs) per dma_gather
                             # (larger gathers crash the SWDGE ucode)
NEG = 0.2
F32 = np.float32

_results_log = []            # BassKernelResults per launch (timing for test.py)


def _batches():
    out = []
    t0 = 0
    while t0 < TB:
        out.append((t0, min(TBAT, TB - t0)))
        t0 += TBAT
    return out


def _install_trace_support():
    """Register the NTFF profile hook this image's antenv lacks, and make
    artifact upload failure non-fatal, so BASS_TRACE reports exec_time_ns."""
    try:
        from antenv.axon_hooks import get_axon_ntff_profile_hook  # noqa: F401
        return
    except ImportError:
        pass
    try:
        import trn_agent_boot.trn_boot as tb
        hook = tb._ntff_profile_via_ctypes("/opt/axon/libaxon_pjrt.so")
        mod = types.ModuleType("antenv.axon_hooks")
        state = {"h": hook}
        mod.get_axon_ntff_profile_hook = lambda: state["h"]
        mod.set_axon_ntff_profile_hook = lambda h: state.__setitem__("h", h)
        sys.modules["antenv.axon_hooks"] = mod
        import antenv
        antenv.axon_hooks = mod
        from concourse import bass_utils as bu
        orig = bu.upload_artifacts

        def safe_upload(tmpdir):
            try:
                return orig(tmpdir)
            except Exception:
                return tmpdir
        bu.upload_artifacts = safe_upload
    except Exception:
        pass


_install_trace_support()


# --------------------------------------------------------------------------
# device programs
# --------------------------------------------------------------------------

def _build_tab_launch(heads, cout, hdt):
    """A-launch: core-sharded table build. One pass over the shard:
    h (hdt, cout cols) and el/er (f32, 2*heads cols)."""
    from concourse import mybir, tile, bacc

    tcols = cout + 2 * heads
    f32 = mybir.dt.float32
    bf16 = mybir.dt.bfloat16
    nc = bacc.Bacc("TRN2", target_bir_lowering=False, debug=False,
                   enable_asserts=False)
    XT = nc.dram_tensor("xt", [NSHPAD, 128], bf16, kind="ExternalInput")
    WE = nc.dram_tensor("we", [CIN, tcols], bf16, kind="ExternalInput")
    TH = nc.dram_tensor("th", [NSHPAD, cout], hdt, kind="ExternalOutput")
    TE = nc.dram_tensor("te", [NSHPAD, 2 * heads], f32, kind="ExternalOutput")

    with tile.TileContext(nc) as tc:
        with tc.tile_pool(name="c", bufs=1) as cpool, \
             tc.tile_pool(name="ps", bufs=8, space="PSUM") as pp:
            we_t = cpool.tile([CIN, tcols], bf16)
            nc.sync.dma_start(we_t[:], WE[:, :])
            xt = cpool.tile([CIN, TB * 128], bf16)
            # XT holds host-transposed tiles: row t*128+d, col n = X[t*128+n, d]
            nc.sync.dma_start(
                xt[:].rearrange("d (t n) -> d t n", t=TB),
                XT[:, :].rearrange("(t d) n -> d t n", d=128))
            ht = cpool.tile([128, TB * cout], hdt)
            et = cpool.tile([128, TB * 2 * heads], f32)
            for i in range(TB):
                ps = pp.tile([128, tcols], f32, tag="ps")
                nc.tensor.matmul(out=ps[:],
                                 lhsT=xt[:, i * 128:(i + 1) * 128],
                                 rhs=we_t[:], start=True, stop=True)
                nc.vector.tensor_copy(
                    out=ht[:, i * cout:(i + 1) * cout], in_=ps[:, :cout])
                nc.vector.tensor_copy(
                    out=et[:, i * 2 * heads:(i + 1) * 2 * heads],
                    in_=ps[:, cout:tcols])
            nc.sync.dma_start(
                TH[:, :].rearrange("(t p) c -> p t c", p=128),
                ht[:].rearrange("p (t c) -> p t c", t=TB))
            nc.sync.dma_start(
                TE[:, :].rearrange("(t p) c -> p t c", p=128),
                et[:].rearrange("p (t c) -> p t c", t=TB))
    nc.compile()
    return nc


def _build_agg_launch(layer, heads, d, kb):
    """B-launch: batched pair-row gather + fused attention + aggregation.

    kb: per-batch uniform slot counts (len=#batches).
    Slot payload: 2*heads groups x 64 cols (256B total). Machine-head
    m = q*heads + h (q = source table-row parity). el/er are per-slot,
    slot-major [p, mh*k2tot] bf16; wrong-parity el = -1e30."""
    from concourse import mybir, tile, bacc

    cout = heads * d            # output width (128 / 64)
    mh = 2 * heads
    scols = mh * d              # slot payload cols (256 fp8 / 128 bf16)
    f32 = mybir.dt.float32
    bf16 = mybir.dt.bfloat16
    gdt = mybir.dt.float8e3 if layer == 1 else bf16
    i16 = mybir.dt.int16
    AT = mybir.ActivationFunctionType
    OP = mybir.AluOpType

    bat = _batches()
    nb_slots = [int(bat[i][1] * kb[i]) for i in range(len(bat))]
    so = np.concatenate([[0], np.cumsum(nb_slots)[:-1]]).astype(int)
    k2tot = int(sum(nb_slots))

    nc = bacc.Bacc("TRN2", target_bir_lowering=False, debug=False,
                   enable_asserts=False, num_swdge_queues=4)
    HT = nc.dram_tensor("ht", [NPAD // 2, scols], gdt, kind="ExternalInput")
    IX = nc.dram_tensor("ix", [128, 8 * k2tot], i16, kind="ExternalInput")
    EL = nc.dram_tensor("el", [128, mh * k2tot], bf16, kind="ExternalInput")
    ER = nc.dram_tensor("er", [128, mh * k2tot], bf16, kind="ExternalInput")
    BI = nc.dram_tensor("bi", [128, cout], f32, kind="ExternalInput")
    if layer == 2:
        H1S = nc.dram_tensor("h1s", [NSHPAD, 128], f32, kind="ExternalInput")
    OUT = nc.dram_tensor("out", [NSHPAD, cout], f32, kind="ExternalOutput")

    qrr = [0]

    with tile.TileContext(nc) as tc:
        with tc.tile_pool(name="c", bufs=1) as cpool, \
             tc.tile_pool(name="pg", bufs=2) as pg, \
             tc.tile_pool(name="pm", bufs=1) as pm, \
             tc.tile_pool(name="pb", bufs=2) as pb, \
             tc.tile_pool(name="sm", bufs=2) as sm:
            ix_t = cpool.tile([128, 8 * k2tot], i16)
            nc.sync.dma_start(ix_t[:], IX[:, :])
            el_t = cpool.tile([128, mh * k2tot], bf16)
            nc.sync.dma_start(el_t[:], EL[:, :])
            er_t = cpool.tile([128, mh * k2tot], bf16)
            nc.sync.dma_start(er_t[:], ER[:, :])
            bi_t = cpool.tile([128, cout], f32)
            nc.sync.dma_start(bi_t[:], BI[:, :])

            def attn_phase(bi_i):
                """Gather + attention front-end for one batch (issued one
                batch ahead so the scalar-engine exps never trail the
                previous batch's normalize in the queue)."""
                t0, tsz = bat[bi_i]
                K = int(kb[bi_i])
                nsl = tsz * K
                ko = int(so[bi_i])
                g = pg.tile([128, nsl * scols], gdt, tag="g")
                b0 = 0
                while b0 < nsl:
                    nb = min(GCHUNK, nsl - b0)
                    nc.gpsimd.dma_gather(
                        out_ap=g[:, b0 * scols:(b0 + nb) * scols].rearrange(
                            "p (b e) -> p b e", e=scols),
                        in_ap=HT[:, :],
                        idxs_ap=ix_t[:, 8 * (ko + b0):8 * (ko + b0 + nb)],
                        num_idxs=nb * 128,
                        num_idxs_reg=nb * 128,
                        elem_size=scols,
                        queue_num=qrr[0] % 4)
                    qrr[0] += 1
                    b0 += nb

                # slot-major: ex = exp(lrelu(el + er)) [p, nsl*mh];
                # exp is monotonic: exp(max(e, .2e)) = max(exp(e), exp(.2e))
                ex = sm.tile([128, nsl * mh], bf16, tag="ex")
                ex2 = sm.tile([128, nsl * mh], bf16, tag="ex2")
                nc.vector.tensor_tensor(
                    out=ex[:], in0=el_t[:, mh * ko:mh * (ko + nsl)],
                    in1=er_t[:, mh * ko:mh * (ko + nsl)], op=OP.add)
                nc.scalar.activation(out=ex2[:], in_=ex[:], func=AT.Exp,
                                     scale=NEG)
                nc.scalar.activation(out=ex[:], in_=ex[:], func=AT.Exp)
                return g, ex, ex2

            state = attn_phase(0)
            for bi_i, (t0, tsz) in enumerate(bat):
                g, ex, ex2 = state
                if bi_i + 1 < len(bat):
                    state = attn_phase(bi_i + 1)
                K = int(kb[bi_i])
                nsl = tsz * K
                ko = int(so[bi_i])
                nc.vector.tensor_tensor(out=ex[:], in0=ex[:], in1=ex2[:],
                                        op=OP.max)

                # denominators: den[p, (t m)] = sum_k ex[p, t, m, k]
                den = sm.tile([128, tsz * mh], f32, tag="den")
                exv = ex[:].rearrange("p (t k m) -> p t k m", k=K,
                                      m=mh).rearrange("p t k m -> p t m k")
                nc.vector.tensor_reduce(
                    out=den[:].rearrange("p (t m) -> p t m", m=mh),
                    in_=exv, axis=mybir.AxisListType.X, op=OP.add)
                rd = sm.tile([128, tsz * heads], f32, tag="rd")
                dv = den[:].rearrange("p (t m) -> p t m", m=mh)
                # fold parity: den_h = den[q0,h] + den[q1,h]
                nc.vector.tensor_tensor(
                    out=rd[:].rearrange("p (t h) -> p t h", h=heads),
                    in0=dv[:, :, 0:heads], in1=dv[:, :, heads:mh], op=OP.add)
                nc.vector.tensor_scalar(
                    out=rd[:], in0=rd[:], scalar1=1e-12, scalar2=None,
                    op0=OP.max)
                nc.vector.reciprocal(out=rd[:], in_=rd[:])

                # messages: p2[p, k, m, c] = g * ex (one broadcast multiply)
                p2 = pm.tile([128, nsl * scols], bf16, tag="p2")
                nc.vector.tensor_tensor(
                    out=p2[:].rearrange("p (k m c) -> p k m c", m=mh, c=d),
                    in0=g[:].rearrange("p (k m c) -> p k m c", m=mh, c=d),
                    in1=ex[:].rearrange("p (k m) -> p k m", m=mh).to_broadcast(
                        [128, nsl, mh, d]),
                    op=OP.mult)

                # slot reduce: fold pair halves into the dead g tile
                # (bitcast to bf16 - contiguous, non-aliased), then a
                # pairwise in-place tree over k
                gb = g[:].bitcast(bf16)          # [128, nsl*scols/2 bf16]
                gf = gb[:, 0:nsl * cout].rearrange("p (k c) -> p k c", c=cout)
                pv = p2[:].rearrange("p (k c) -> p k c", c=scols)
                nc.vector.tensor_tensor(
                    out=gf, in0=pv[:, :, 0:cout],
                    in1=pv[:, :, cout:scols], op=OP.add)
                gt = gb[:, 0:nsl * cout].rearrange("p (t k c) -> p t k c",
                                                   k=K, c=cout)
                kk = K
                while kk > 1:
                    hh2 = kk // 2
                    nc.vector.tensor_tensor(
                        out=gt[:, :, 0:hh2, :],
                        in0=gt[:, :, 0:hh2, :],
                        in1=gt[:, :, kk - hh2:kk, :], op=OP.add)
                    kk -= hh2
                nv = gt[:, :, 0, :]          # [p, t, cout] bf16
                o = sm.tile([128, tsz * cout], f32, tag="o")
                ov = o[:].rearrange("p (t c) -> p t c", c=cout)
                sc = sm.tile([128, tsz * cout], f32, tag="sc")
                scv = sc[:].rearrange("p (t c) -> p t c", c=cout)

                # normalize on the scalar engine (per-partition scale AP)
                rv = rd[:].rearrange("p (t h) -> p t h", h=heads)
                for ti in range(tsz):
                    for hh in range(heads):
                        nc.scalar.mul(
                            ov[:, ti, hh * d:(hh + 1) * d],
                            nv[:, ti, hh * d:(hh + 1) * d],
                            rv[:, ti, hh:hh + 1])
                nc.vector.tensor_tensor(
                    out=ov[:], in0=ov[:],
                    in1=bi_t[:, None, :].to_broadcast([128, tsz, cout]),
                    op=OP.add)
                if layer == 1:
                    # elu(x) = max(x, exp(min(x,0)) - 1)
                    nc.vector.tensor_scalar(out=sc[:], in0=o[:], scalar1=0.0,
                                            scalar2=None, op0=OP.min)
                    nc.scalar.activation(out=sc[:], in_=sc[:], func=AT.Exp)
                    nc.scalar.activation(out=sc[:], in_=sc[:], func=AT.Copy,
                                         bias=-1.0)
                    nc.vector.tensor_tensor(out=o[:], in0=o[:], in1=sc[:],
                                            op=OP.max)
                else:
                    # out = 0.5*o + 0.25*(h1[:,0:64] + h1[:,64:128])
                    h1t = pb.tile([128, tsz * 128], f32, tag="h1t")
                    nc.sync.dma_start(
                        h1t[:].rearrange("p (t c) -> p t c", t=tsz),
                        H1S[t0 * 128:(t0 + tsz) * 128, :].rearrange(
                            "(t p) c -> p t c", p=128))
                    hv = h1t[:].rearrange("p (t c) -> p t c", c=128)
                    nc.vector.tensor_tensor(out=scv[:], in0=hv[:, :, 0:d],
                                            in1=hv[:, :, d:2 * d], op=OP.add)
                    nc.vector.tensor_scalar(out=sc[:], in0=sc[:], scalar1=0.25,
                                            scalar2=None, op0=OP.mult)
                    nc.vector.tensor_scalar(out=o[:], in0=o[:], scalar1=0.5,
                                            scalar2=None, op0=OP.mult)
                    nc.vector.tensor_tensor(out=o[:], in0=o[:], in1=sc[:],
                                            op=OP.add)
                nc.sync.dma_start(
                    OUT[t0 * 128:(t0 + tsz) * 128, :].rearrange(
                        "(t p) c -> p t c", p=128),
                    o[:].rearrange("p (t c) -> p t c", t=tsz))
    nc.compile()
    return nc


# --------------------------------------------------------------------------
# host-side graph prep (pure routing: shard / sort / index tables)
# --------------------------------------------------------------------------

def _prep_graph(src, dst):
    """Degree-sorted round-robin sharding + per-batch uniform-K slot grid.

    Returns ranks, pos, kb (per-batch K), k2tot, and per-core
    (slot_src [128, k2tot] source NODE id, -1 pad;
     slot_dst [128, k2tot] global padded dst TABLE row, -1 pad)."""
    deg = np.bincount(dst, minlength=N)
    ranks = np.argsort(-deg, kind="stable").astype(np.int64)
    pos = np.empty(N, np.int64)
    pos[ranks] = np.arange(N)
    ec = (pos[dst] % NCORES).astype(np.int64)
    ej = (pos[dst] // NCORES).astype(np.int64)
    src = src.astype(np.int64)

    bat = _batches()
    sdeg = deg[ranks]
    kat = np.zeros(TB, np.int64)
    for t in range(TB):
        sl = sdeg[t * 1024:(t + 1) * 1024]
        kat[t] = max(1, int(sl.max()) if len(sl) else 1)
    kb = np.array([int(kat[t0:t0 + tsz].max()) for t0, tsz in bat], np.int64)
    nb_slots = np.array([bat[i][1] * kb[i] for i in range(len(bat))], np.int64)
    so = np.concatenate([[0], np.cumsum(nb_slots)[:-1]]).astype(np.int64)
    k2tot = int(nb_slots.sum())

    tile_bi = np.repeat(np.arange(len(bat)), [tsz for _, tsz in bat])
    tile_i = np.concatenate([np.arange(tsz) for _, tsz in bat])

    slot_src, slot_dst = [], []
    for c in range(NCORES):
        m = ec == c
        js, ss = ej[m], src[m]
        order = np.argsort(js * (2 * N) + ss, kind="stable")
        js, ss = js[order], ss[order]
        cnt = np.bincount(js, minlength=NSHPAD)
        starts = np.concatenate([[0], np.cumsum(cnt)[:-1]])
        within = np.arange(len(js)) - starts[js]
        tile_id = js // 128
        p = js % 128
        col = so[tile_bi[tile_id]] + tile_i[tile_id] * kb[tile_bi[tile_id]] \
            + within
        arr = np.full((128, k2tot), -1, np.int64)
        dstn = np.full((128, k2tot), -1, np.int64)
        arr[p, col] = ss
        dstn[p, col] = c * NSHPAD + js
        slot_src.append(arr)
        slot_dst.append(dstn)
    return ranks, pos, kb, k2tot, slot_src, slot_dst


def _wrap_idx(idx16):
    """Wrapped int16 index array [128, 8*k2tot]: a gather over slot cols
    [k0, k0+nb) reads cols 8*k0 .. 8*(k0+nb); idx i = k*128+p sits at
    [(p%16), 8*k + p//16]."""
    k2tot = idx16.shape[1]
    out = np.zeros((16, 8 * k2tot), np.int16)
    p = np.arange(128)
    for k in range(k2tot):
        out[p % 16, 8 * k + p // 16] = idx16[:, k]
    return np.tile(out, (8, 1))


def _xt_shard(xtab, c):
    """Host-transposed tiles of the table-order features:
    row t*128+d, col n = xtab[c*NSHPAD + t*128 + n, d]  (bf16)."""
    import ml_dtypes
    bf16 = np.dtype(ml_dtypes.bfloat16)
    xp = xtab[c * NSHPAD:(c + 1) * NSHPAD]
    return np.ascontiguousarray(
        xp.reshape(TB, 128, CIN).transpose(0, 2, 1).astype(bf16)
    ).reshape(NSHPAD, CIN)


def _run(nc, in_maps):
    from concourse.bass_utils import run_bass_kernel_spmd
    trace = bool(os.environ.get("GAT_TRACE"))
    res = run_bass_kernel_spmd(nc, in_maps, list(range(NCORES)), trace=trace)
    _results_log.append(res)
    return res.results


def _wext(W, al, ar, heads, d):
    import ml_dtypes
    bf16 = np.dtype(ml_dtypes.bfloat16)
    A = np.zeros((heads * d, heads), F32)
    R = np.zeros((heads * d, heads), F32)
    for h in range(heads):
        A[h * d:(h + 1) * d, h] = al[h]
        R[h * d:(h + 1) * d, h] = ar[h]
    return np.ascontiguousarray(np.hstack([W, W @ A, W @ R]).astype(bf16))


_cache = {}


def kernel(feature, src, dst, W1, al1, ar1, b1, W2, al2, ar2, b2):
    import ml_dtypes  # bfloat16 / fp8 numpy dtypes
    bf16 = np.dtype(ml_dtypes.bfloat16)

    feature = np.asarray(feature, F32)
    src = np.asarray(src, np.int32)
    dst = np.asarray(dst, np.int32)
    W1, al1, ar1, b1 = (np.asarray(a, F32) for a in (W1, al1, ar1, b1))
    W2, al2, ar2, b2 = (np.asarray(a, F32) for a in (W2, al2, ar2, b2))

    ranks, pos, kb, k2tot, slot_src, slot_dst = _prep_graph(src, dst)
    key = tuple(kb)
    if key not in _cache:
        from concourse import mybir
        _cache[key] = (
            _build_tab_launch(2, 128, mybir.dt.float8e3),
            _build_tab_launch(1, 64, mybir.dt.bfloat16),
            _build_agg_launch(1, 2, 64, kb),
            _build_agg_launch(2, 1, 64, kb),
        )
    nc_t1, nc_t2, nc_b1, nc_b2 = _cache[key]

    # node id -> padded global table row (core-major shards, round-robin)
    pos_pad = (pos % NCORES) * NSHPAD + pos // NCORES

    pads = [s < 0 for s in slot_src]
    # source table row per slot (pads -> row 0)
    srow = [np.where(p, 0, pos_pad[np.where(p, 0, s)])
            for s, p in zip(slot_src, pads)]
    idxw = [_wrap_idx((r // 2).astype(np.int16)) for r in srow]

    def layer(lnum, xtab, heads, d, W, al, ar, b, nc_tab, nc_agg,
              h1_shards=None):
        cout = heads * d
        mh = 2 * heads
        We = _wext(W, al, ar, heads, d)
        res_t = _run(nc_tab, [dict(xt=_xt_shard(xtab, c), we=We)
                              for c in range(NCORES)])
        ht = np.ascontiguousarray(np.concatenate(
            [np.asarray(res_t[c]["th"]) for c in range(NCORES)], 0))
        ht = ht.reshape(NPAD // 2, 2 * cout)       # pair rows
        te = np.concatenate([np.asarray(res_t[c]["te"])
                             for c in range(NCORES)], 0)   # [NPAD, 2*heads]
        el_nodes = np.concatenate(
            [te[:, :heads], np.full((1, heads), -1e30, F32)], 0)
        er_nodes = np.concatenate(
            [te[:, heads:2 * heads], np.zeros((1, heads), F32)], 0)
        bi = np.ascontiguousarray(np.tile(b[None, :], (128, 1)).astype(F32))

        in_maps = []
        for c in range(NCORES):
            pad = pads[c]
            pr = srow[c]                              # source table row
            sl = np.where(pad, NPAD, pr)
            el_s = el_nodes[sl]                       # [128, k2tot, heads]
            er_s = er_nodes[np.where(slot_dst[c] < 0, NPAD, slot_dst[c])]
            parity = (pr % 2).astype(np.int64)
            el4 = np.full((128, k2tot, mh), -1e30, F32)
            er4 = np.zeros((128, k2tot, mh), F32)
            for q in range(2):
                for h in range(heads):
                    mm = q * heads + h
                    el4[:, :, mm] = np.where(parity == q,
                                             el_s[:, :, h], -1e30)
                    er4[:, :, mm] = er_s[:, :, h]
            el4 = np.ascontiguousarray(
                el4.reshape(128, mh * k2tot).astype(bf16))
            er4 = np.ascontiguousarray(
                er4.reshape(128, mh * k2tot).astype(bf16))
            m = dict(ht=ht, ix=idxw[c], el=el4, er=er4, bi=bi)
            if lnum == 2:
                m["h1s"] = h1_shards[c]
            in_maps.append(m)
        res = _run(nc_agg, in_maps)
        return [np.ascontiguousarray(np.asarray(res[c]["out"], F32))
                for c in range(NCORES)]

    # features reordered into table (padded, core-major) order
    xtab = np.zeros((NPAD, CIN), F32)
    xtab[pos_pad] = feature
    h1_shards = layer(1, xtab, 2, 64, W1, al1, ar1, b1, nc_t1, nc_b1)
    h1_tab = np.concatenate(h1_shards, 0)           # [NPAD, 128] table order

    out_shards = layer(2, h1_tab, 1, 64, W2, al2, ar2, b2, nc_t2, nc_b2,
                       h1_shards)
    out_tab = np.concatenate(out_shards, 0)
    return np.ascontiguousarray(out_tab[pos_pad])


# revision 19
# speedup vs baseline: 1.3257x; 1.3257x over previous
"""GAT 2-layer kernel for Trainium2, 8 NeuronCores (Bass/Tile).

Strategy (graph/data parallel per the sharding hint):
  - Nodes are degree-sorted and dealt round-robin to the 8 cores; each core
    owns the edges whose dst it owns, so edge-softmax and the weighted
    aggregation are core-local.
  - Per GAT layer, two SPMD launches:
      A-launch: table build, sharded by node id - core c computes rows of
        T = X @ [W | W@al | W@ar]; h is written as a packed table, el/er as
        a small f32 side table.
      B-launch: batches of dst-node tiles; per batch, gather all in-edge
        source rows with dma_gather, then a fused attention pipeline: one
        add (el+er), leaky-relu+exp on the scalar engine, one broadcast
        multiply, one slot reduce, fold, normalize, bias, activation.
  - The host only routes bytes between launches (shard/gather/concat); all
    arithmetic runs on device.

Table packing: rows hold a PAIR of nodes in 256 bytes (the dma_gather
minimum element), idx = table_row//2, so the table footprint is halved
(6.4MB) and HBM random reads stay fast:
  layer 1: 2 x 128 fp8(e3m4) cols; layer 2: 2 x 64 bf16 cols.
The wrong pair-mate is masked by el = -1e30 => softmax weight 0. Machine
heads m = parity*heads + h (4 for layer 1, 2 for layer 2).
"""

import os
import sys
import types
import numpy as np

sys.path.insert(0, "/opt/trn_rl_repo")

N = 50000
E = 800000
CIN = 128
NCORES = 8
NSH = N // NCORES            # 6250 nodes per core
TB = (NSH + 127) // 128      # 49 dst tiles per core
NSHPAD = TB * 128            # 6272
NPAD = NCORES * NSHPAD       # 50176 table rows
TBAT = 4                     # dst tiles per batch in the B-launch
GCHUNK = 8                   # max slots (1024 idxs) per dma_gather
                             # (larger gathers crash the SWDGE ucode)
NEG = 0.2
F32 = np.float32

_results_log = []            # BassKernelResults per launch (timing for test.py)


def _batches():
    # the 4 highest-degree tiles go solo (exact K, no cross-tile padding);
    # the rest in groups of TBAT
    out = [(0, 1), (1, 1), (2, 1), (3, 1)]
    t0 = 4
    while t0 < TB:
        out.append((t0, min(TBAT, TB - t0)))
        t0 += TBAT
    return out


def _install_trace_support():
    """Register the NTFF profile hook this image's antenv lacks, and make
    artifact upload failure non-fatal, so BASS_TRACE reports exec_time_ns."""
    try:
        from antenv.axon_hooks import get_axon_ntff_profile_hook  # noqa: F401
        return
    except ImportError:
        pass
    try:
        import trn_agent_boot.trn_boot as tb
        hook = tb._ntff_profile_via_ctypes("/opt/axon/libaxon_pjrt.so")
        mod = types.ModuleType("antenv.axon_hooks")
        state = {"h": hook}
        mod.get_axon_ntff_profile_hook = lambda: state["h"]
        mod.set_axon_ntff_profile_hook = lambda h: state.__setitem__("h", h)
        sys.modules["antenv.axon_hooks"] = mod
        import antenv
        antenv.axon_hooks = mod
        from concourse import bass_utils as bu
        orig = bu.upload_artifacts

        def safe_upload(tmpdir):
            try:
                return orig(tmpdir)
            except Exception:
                return tmpdir
        bu.upload_artifacts = safe_upload
    except Exception:
        pass


_install_trace_support()


# --------------------------------------------------------------------------
# device programs
# --------------------------------------------------------------------------

def _build_tab_launch(heads, cout, hdt):
    """A-launch: core-sharded table build. One pass over the shard:
    h (hdt, cout cols) and el/er (f32, 2*heads cols)."""
    from concourse import mybir, tile, bacc

    tcols = cout + 2 * heads
    f32 = mybir.dt.float32
    bf16 = mybir.dt.bfloat16
    nc = bacc.Bacc("TRN2", target_bir_lowering=False, debug=False,
                   enable_asserts=False)
    XT = nc.dram_tensor("xt", [NSHPAD, 128], bf16, kind="ExternalInput")
    WE = nc.dram_tensor("we", [CIN, tcols], bf16, kind="ExternalInput")
    TH = nc.dram_tensor("th", [NSHPAD, cout], hdt, kind="ExternalOutput")
    TE = nc.dram_tensor("te", [NSHPAD, 2 * heads], f32, kind="ExternalOutput")

    with tile.TileContext(nc) as tc:
        with tc.tile_pool(name="c", bufs=1) as cpool, \
             tc.tile_pool(name="ps", bufs=8, space="PSUM") as pp:
            we_t = cpool.tile([CIN, tcols], bf16)
            nc.sync.dma_start(we_t[:], WE[:, :])
            xt = cpool.tile([CIN, TB * 128], bf16)
            # XT holds host-transposed tiles: row t*128+d, col n = X[t*128+n, d]
            nc.sync.dma_start(
                xt[:].rearrange("d (t n) -> d t n", t=TB),
                XT[:, :].rearrange("(t d) n -> d t n", d=128))
            ht = cpool.tile([128, TB * cout], hdt)
            et = cpool.tile([128, TB * 2 * heads], f32)
            for i in range(TB):
                ps = pp.tile([128, tcols], f32, tag="ps")
                nc.tensor.matmul(out=ps[:],
                                 lhsT=xt[:, i * 128:(i + 1) * 128],
                                 rhs=we_t[:], start=True, stop=True)
                nc.vector.tensor_copy(
                    out=ht[:, i * cout:(i + 1) * cout], in_=ps[:, :cout])
                nc.vector.tensor_copy(
                    out=et[:, i * 2 * heads:(i + 1) * 2 * heads],
                    in_=ps[:, cout:tcols])
            nc.sync.dma_start(
                TH[:, :].rearrange("(t p) c -> p t c", p=128),
                ht[:].rearrange("p (t c) -> p t c", t=TB))
            nc.sync.dma_start(
                TE[:, :].rearrange("(t p) c -> p t c", p=128),
                et[:].rearrange("p (t c) -> p t c", t=TB))
    nc.compile()
    return nc


def _build_agg_launch(layer, heads, d, kb):
    """B-launch: batched pair-row gather + fused attention + aggregation.

    kb: per-batch uniform slot counts (len=#batches).
    Slot payload: 2*heads groups x 64 cols (256B total). Machine-head
    m = q*heads + h (q = source table-row parity). el/er are per-slot,
    slot-major [p, mh*k2tot] bf16; wrong-parity el = -1e30."""
    from concourse import mybir, tile, bacc

    cout = heads * d            # output width (128 / 64)
    mh = 2 * heads
    scols = mh * d              # slot payload cols (256 fp8 / 128 bf16)
    f32 = mybir.dt.float32
    bf16 = mybir.dt.bfloat16
    gdt = mybir.dt.float8e3 if layer == 1 else bf16
    i16 = mybir.dt.int16
    AT = mybir.ActivationFunctionType
    OP = mybir.AluOpType

    bat = _batches()
    nb_slots = [int(bat[i][1] * kb[i]) for i in range(len(bat))]
    so = np.concatenate([[0], np.cumsum(nb_slots)[:-1]]).astype(int)
    k2tot = int(sum(nb_slots))

    nc = bacc.Bacc("TRN2", target_bir_lowering=False, debug=False,
                   enable_asserts=False, num_swdge_queues=4)
    HT = nc.dram_tensor("ht", [NPAD // 2, scols], gdt, kind="ExternalInput")
    IX = nc.dram_tensor("ix", [128, 8 * k2tot], i16, kind="ExternalInput")
    EL = nc.dram_tensor("el", [128, mh * k2tot], bf16, kind="ExternalInput")
    ER = nc.dram_tensor("er", [128, mh * k2tot], bf16, kind="ExternalInput")
    BI = nc.dram_tensor("bi", [128, cout], f32, kind="ExternalInput")
    if layer == 2:
        H1S = nc.dram_tensor("h1s", [NSHPAD, 128], f32, kind="ExternalInput")
    OUT = nc.dram_tensor("out", [NSHPAD, cout], f32, kind="ExternalOutput")

    qrr = [0]

    with tile.TileContext(nc) as tc:
        with tc.tile_pool(name="c", bufs=1) as cpool, \
             tc.tile_pool(name="pg", bufs=3) as pg, \
             tc.tile_pool(name="pm", bufs=1) as pm, \
             tc.tile_pool(name="pf", bufs=2) as pf_pool, \
             tc.tile_pool(name="pb", bufs=2) as pb, \
             tc.tile_pool(name="sm", bufs=2) as sm:
            ix_t = cpool.tile([128, 8 * k2tot], i16)
            nc.sync.dma_start(ix_t[:], IX[:, :])
            el_t = cpool.tile([128, mh * k2tot], bf16)
            nc.sync.dma_start(el_t[:], EL[:, :])
            er_t = cpool.tile([128, mh * k2tot], bf16)
            nc.sync.dma_start(er_t[:], ER[:, :])
            bi_t = cpool.tile([128, cout], f32)
            nc.sync.dma_start(bi_t[:], BI[:, :])

            def attn_phase(bi_i):
                """Gather + attention front-end for one batch (issued one
                batch ahead so the scalar-engine exps never trail the
                previous batch's normalize in the queue)."""
                t0, tsz = bat[bi_i]
                K = int(kb[bi_i])
                nsl = tsz * K
                ko = int(so[bi_i])
                g = pg.tile([128, nsl * scols], gdt, tag="g")
                b0 = 0
                while b0 < nsl:
                    nb = min(GCHUNK, nsl - b0)
                    nc.gpsimd.dma_gather(
                        out_ap=g[:, b0 * scols:(b0 + nb) * scols].rearrange(
                            "p (b e) -> p b e", e=scols),
                        in_ap=HT[:, :],
                        idxs_ap=ix_t[:, 8 * (ko + b0):8 * (ko + b0 + nb)],
                        num_idxs=nb * 128,
                        num_idxs_reg=nb * 128,
                        elem_size=scols,
                        queue_num=qrr[0] % 4)
                    qrr[0] += 1
                    b0 += nb

                # slot-major: ex = exp(lrelu(el + er)) [p, nsl*mh];
                # exp is monotonic: exp(max(e, .2e)) = max(exp(e), exp(.2e))
                ex = sm.tile([128, nsl * mh], bf16, tag="ex")
                ex2 = sm.tile([128, nsl * mh], bf16, tag="ex2")
                nc.vector.tensor_tensor(
                    out=ex[:], in0=el_t[:, mh * ko:mh * (ko + nsl)],
                    in1=er_t[:, mh * ko:mh * (ko + nsl)], op=OP.add)
                nc.scalar.activation(out=ex2[:], in_=ex[:], func=AT.Exp,
                                     scale=NEG)
                nc.scalar.activation(out=ex[:], in_=ex[:], func=AT.Exp)
                return g, ex, ex2

            state = attn_phase(0)
            for bi_i, (t0, tsz) in enumerate(bat):
                g, ex, ex2 = state
                if bi_i + 1 < len(bat):
                    state = attn_phase(bi_i + 1)
                K = int(kb[bi_i])
                nsl = tsz * K
                ko = int(so[bi_i])
                nc.vector.tensor_tensor(out=ex[:], in0=ex[:], in1=ex2[:],
                                        op=OP.max)

                # denominators: den[p, (t m)] = sum_k ex[p, t, m, k]
                den = sm.tile([128, tsz * mh], f32, tag="den")
                exv = ex[:].rearrange("p (t k m) -> p t k m", k=K,
                                      m=mh).rearrange("p t k m -> p t m k")
                nc.vector.tensor_reduce(
                    out=den[:].rearrange("p (t m) -> p t m", m=mh),
                    in_=exv, axis=mybir.AxisListType.X, op=OP.add)
                rd = sm.tile([128, tsz * heads], f32, tag="rd")
                dv = den[:].rearrange("p (t m) -> p t m", m=mh)
                # fold parity: den_h = den[q0,h] + den[q1,h]
                nc.vector.tensor_tensor(
                    out=rd[:].rearrange("p (t h) -> p t h", h=heads),
                    in0=dv[:, :, 0:heads], in1=dv[:, :, heads:mh], op=OP.add)
                nc.vector.tensor_scalar(
                    out=rd[:], in0=rd[:], scalar1=1e-12, scalar2=None,
                    op0=OP.max)
                nc.vector.reciprocal(out=rd[:], in_=rd[:])

                # messages: p2[p, k, m, c] = g * ex (one broadcast multiply)
                p2 = pm.tile([128, nsl * scols], bf16, tag="p2")
                nc.vector.tensor_tensor(
                    out=p2[:].rearrange("p (k m c) -> p k m c", m=mh, c=d),
                    in0=g[:].rearrange("p (k m c) -> p k m c", m=mh, c=d),
                    in1=ex[:].rearrange("p (k m) -> p k m", m=mh).to_broadcast(
                        [128, nsl, mh, d]),
                    op=OP.mult)

                # slot reduce: fold pair halves into a packed contiguous
                # buffer (p2 frees right away, g right after the mult),
                # then a fully-contiguous pairwise tree over k
                pf = pf_pool.tile([128, nsl * cout], bf16, tag="pf")
                pv = p2[:].rearrange("p (k c) -> p k c", c=scols)
                nc.vector.tensor_tensor(
                    out=pf[:].rearrange("p (k c) -> p k c", c=cout),
                    in0=pv[:, :, 0:cout],
                    in1=pv[:, :, cout:scols], op=OP.add)
                pt = pf[:].rearrange("p (t k c) -> p t k c", k=K, c=cout)
                kk = K
                while kk > 1:
                    hh2 = kk // 2
                    nc.vector.tensor_tensor(
                        out=pt[:, :, 0:hh2, :],
                        in0=pt[:, :, 0:hh2, :],
                        in1=pt[:, :, kk - hh2:kk, :], op=OP.add)
                    kk -= hh2
                nv = pt[:, :, 0, :]          # [p, t, cout] bf16
                o = sm.tile([128, tsz * cout], f32, tag="o")
                ov = o[:].rearrange("p (t c) -> p t c", c=cout)
                sc = sm.tile([128, tsz * cout], f32, tag="sc")
                scv = sc[:].rearrange("p (t c) -> p t c", c=cout)

                # normalize on the scalar engine (per-partition scale AP)
                rv = rd[:].rearrange("p (t h) -> p t h", h=heads)
                for ti in range(tsz):
                    for hh in range(heads):
                        nc.scalar.mul(
                            ov[:, ti, hh * d:(hh + 1) * d],
                            nv[:, ti, hh * d:(hh + 1) * d],
                            rv[:, ti, hh:hh + 1])
                nc.vector.tensor_tensor(
                    out=ov[:], in0=ov[:],
                    in1=bi_t[:, None, :].to_broadcast([128, tsz, cout]),
                    op=OP.add)
                if layer == 1:
                    # elu(x) = max(x, exp(min(x,0)) - 1)
                    nc.vector.tensor_scalar(out=sc[:], in0=o[:], scalar1=0.0,
                                            scalar2=None, op0=OP.min)
                    nc.scalar.activation(out=sc[:], in_=sc[:], func=AT.Exp)
                    nc.scalar.activation(out=sc[:], in_=sc[:], func=AT.Copy,
                                         bias=-1.0)
                    nc.vector.tensor_tensor(out=o[:], in0=o[:], in1=sc[:],
                                            op=OP.max)
                else:
                    # out = 0.5*o + 0.25*(h1[:,0:64] + h1[:,64:128])
                    h1t = pb.tile([128, tsz * 128], f32, tag="h1t")
                    nc.sync.dma_start(
                        h1t[:].rearrange("p (t c) -> p t c", t=tsz),
                        H1S[t0 * 128:(t0 + tsz) * 128, :].rearrange(
                            "(t p) c -> p t c", p=128))
                    hv = h1t[:].rearrange("p (t c) -> p t c", c=128)
                    nc.vector.tensor_tensor(out=scv[:], in0=hv[:, :, 0:d],
                                            in1=hv[:, :, d:2 * d], op=OP.add)
                    nc.vector.tensor_scalar(out=sc[:], in0=sc[:], scalar1=0.25,
                                            scalar2=None, op0=OP.mult)
                    nc.vector.tensor_scalar(out=o[:], in0=o[:], scalar1=0.5,
                                            scalar2=None, op0=OP.mult)
                    nc.vector.tensor_tensor(out=o[:], in0=o[:], in1=sc[:],
                                            op=OP.add)
                nc.sync.dma_start(
                    OUT[t0 * 128:(t0 + tsz) * 128, :].rearrange(
                        "(t p) c -> p t c", p=128),
                    o[:].rearrange("p (t c) -> p t c", t=tsz))
    nc.compile()
    return nc


# --------------------------------------------------------------------------
# host-side graph prep (pure routing: shard / sort / index tables)
# --------------------------------------------------------------------------

def _prep_graph(src, dst):
    """Degree-sorted round-robin sharding + per-batch uniform-K slot grid.

    Returns ranks, pos, kb (per-batch K), k2tot, and per-core
    (slot_src [128, k2tot] source NODE id, -1 pad;
     slot_dst [128, k2tot] global padded dst TABLE row, -1 pad)."""
    deg = np.bincount(dst, minlength=N)
    ranks = np.argsort(-deg, kind="stable").astype(np.int64)
    pos = np.empty(N, np.int64)
    pos[ranks] = np.arange(N)
    ec = (pos[dst] % NCORES).astype(np.int64)
    ej = (pos[dst] // NCORES).astype(np.int64)
    src = src.astype(np.int64)

    bat = _batches()
    sdeg = deg[ranks]
    kat = np.zeros(TB, np.int64)
    for t in range(TB):
        sl = sdeg[t * 1024:(t + 1) * 1024]
        kat[t] = max(1, int(sl.max()) if len(sl) else 1)
    kb = np.array([int(kat[t0:t0 + tsz].max()) for t0, tsz in bat], np.int64)
    nb_slots = np.array([bat[i][1] * kb[i] for i in range(len(bat))], np.int64)
    so = np.concatenate([[0], np.cumsum(nb_slots)[:-1]]).astype(np.int64)
    k2tot = int(nb_slots.sum())

    tile_bi = np.repeat(np.arange(len(bat)), [tsz for _, tsz in bat])
    tile_i = np.concatenate([np.arange(tsz) for _, tsz in bat])

    slot_src, slot_dst = [], []
    for c in range(NCORES):
        m = ec == c
        js, ss = ej[m], src[m]
        order = np.argsort(js * (2 * N) + ss, kind="stable")
        js, ss = js[order], ss[order]
        cnt = np.bincount(js, minlength=NSHPAD)
        starts = np.concatenate([[0], np.cumsum(cnt)[:-1]])
        within = np.arange(len(js)) - starts[js]
        tile_id = js // 128
        p = js % 128
        col = so[tile_bi[tile_id]] + tile_i[tile_id] * kb[tile_bi[tile_id]] \
            + within
        arr = np.full((128, k2tot), -1, np.int64)
        dstn = np.full((128, k2tot), -1, np.int64)
        arr[p, col] = ss
        dstn[p, col] = c * NSHPAD + js
        slot_src.append(arr)
        slot_dst.append(dstn)
    return ranks, pos, kb, k2tot, slot_src, slot_dst


def _wrap_idx(idx16):
    """Wrapped int16 index array [128, 8*k2tot]: a gather over slot cols
    [k0, k0+nb) reads cols 8*k0 .. 8*(k0+nb); idx i = k*128+p sits at
    [(p%16), 8*k + p//16]."""
    k2tot = idx16.shape[1]
    out = np.zeros((16, 8 * k2tot), np.int16)
    p = np.arange(128)
    for k in range(k2tot):
        out[p % 16, 8 * k + p // 16] = idx16[:, k]
    return np.tile(out, (8, 1))


def _xt_shard(xtab, c):
    """Host-transposed tiles of the table-order features:
    row t*128+d, col n = xtab[c*NSHPAD + t*128 + n, d]  (bf16)."""
    import ml_dtypes
    bf16 = np.dtype(ml_dtypes.bfloat16)
    xp = xtab[c * NSHPAD:(c + 1) * NSHPAD]
    return np.ascontiguousarray(
        xp.reshape(TB, 128, CIN).transpose(0, 2, 1).astype(bf16)
    ).reshape(NSHPAD, CIN)


def _run(nc, in_maps):
    from concourse.bass_utils import run_bass_kernel_spmd
    trace = bool(os.environ.get("GAT_TRACE"))
    res = run_bass_kernel_spmd(nc, in_maps, list(range(NCORES)), trace=trace)
    _results_log.append(res)
    return res.results


def _wext(W, al, ar, heads, d):
    import ml_dtypes
    bf16 = np.dtype(ml_dtypes.bfloat16)
    A = np.zeros((heads * d, heads), F32)
    R = np.zeros((heads * d, heads), F32)
    for h in range(heads):
        A[h * d:(h + 1) * d, h] = al[h]
        R[h * d:(h + 1) * d, h] = ar[h]
    return np.ascontiguousarray(np.hstack([W, W @ A, W @ R]).astype(bf16))


_cache = {}


def kernel(feature, src, dst, W1, al1, ar1, b1, W2, al2, ar2, b2):
    import ml_dtypes  # bfloat16 / fp8 numpy dtypes
    bf16 = np.dtype(ml_dtypes.bfloat16)

    feature = np.asarray(feature, F32)
    src = np.asarray(src, np.int32)
    dst = np.asarray(dst, np.int32)
    W1, al1, ar1, b1 = (np.asarray(a, F32) for a in (W1, al1, ar1, b1))
    W2, al2, ar2, b2 = (np.asarray(a, F32) for a in (W2, al2, ar2, b2))

    ranks, pos, kb, k2tot, slot_src, slot_dst = _prep_graph(src, dst)
    key = tuple(kb)
    if key not in _cache:
        from concourse import mybir
        _cache[key] = (
            _build_tab_launch(2, 128, mybir.dt.float8e3),
            _build_tab_launch(1, 64, mybir.dt.bfloat16),
            _build_agg_launch(1, 2, 64, kb),
            _build_agg_launch(2, 1, 64, kb),
        )
    nc_t1, nc_t2, nc_b1, nc_b2 = _cache[key]

    # node id -> padded global table row (core-major shards, round-robin)
    pos_pad = (pos % NCORES) * NSHPAD + pos // NCORES

    pads = [s < 0 for s in slot_src]
    # source table row per slot (pads -> row 0)
    srow = [np.where(p, 0, pos_pad[np.where(p, 0, s)])
            for s, p in zip(slot_src, pads)]
    idxw = [_wrap_idx((r // 2).astype(np.int16)) for r in srow]

    def layer(lnum, xtab, heads, d, W, al, ar, b, nc_tab, nc_agg,
              h1_shards=None):
        cout = heads * d
        mh = 2 * heads
        We = _wext(W, al, ar, heads, d)
        res_t = _run(nc_tab, [dict(xt=_xt_shard(xtab, c), we=We)
                              for c in range(NCORES)])
        ht = np.ascontiguousarray(np.concatenate(
            [np.asarray(res_t[c]["th"]) for c in range(NCORES)], 0))
        ht = ht.reshape(NPAD // 2, 2 * cout)       # pair rows
        te = np.concatenate([np.asarray(res_t[c]["te"])
                             for c in range(NCORES)], 0)   # [NPAD, 2*heads]
        el_nodes = np.concatenate(
            [te[:, :heads], np.full((1, heads), -1e30, F32)], 0)
        er_nodes = np.concatenate(
            [te[:, heads:2 * heads], np.zeros((1, heads), F32)], 0)
        bi = np.ascontiguousarray(np.tile(b[None, :], (128, 1)).astype(F32))

        in_maps = []
        for c in range(NCORES):
            pad = pads[c]
            pr = srow[c]                              # source table row
            sl = np.where(pad, NPAD, pr)
            el_s = el_nodes[sl]                       # [128, k2tot, heads]
            er_s = er_nodes[np.where(slot_dst[c] < 0, NPAD, slot_dst[c])]
            parity = (pr % 2).astype(np.int64)
            el4 = np.full((128, k2tot, mh), -1e30, F32)
            er4 = np.zeros((128, k2tot, mh), F32)
            for q in range(2):
                for h in range(heads):
                    mm = q * heads + h
                    el4[:, :, mm] = np.where(parity == q,
                                             el_s[:, :, h], -1e30)
                    er4[:, :, mm] = er_s[:, :, h]
            el4 = np.ascontiguousarray(
                el4.reshape(128, mh * k2tot).astype(bf16))
            er4 = np.ascontiguousarray(
                er4.reshape(128, mh * k2tot).astype(bf16))
            m = dict(ht=ht, ix=idxw[c], el=el4, er=er4, bi=bi)
            if lnum == 2:
                m["h1s"] = h1_shards[c]
            in_maps.append(m)
        res = _run(nc_agg, in_maps)
        return [np.ascontiguousarray(np.asarray(res[c]["out"], F32))
                for c in range(NCORES)]

    # features reordered into table (padded, core-major) order
    xtab = np.zeros((NPAD, CIN), F32)
    xtab[pos_pad] = feature
    h1_shards = layer(1, xtab, 2, 64, W1, al1, ar1, b1, nc_t1, nc_b1)
    h1_tab = np.concatenate(h1_shards, 0)           # [NPAD, 128] table order

    out_shards = layer(2, h1_tab, 1, 64, W2, al2, ar2, b2, nc_t2, nc_b2,
                       h1_shards)
    out_tab = np.concatenate(out_shards, 0)
    return np.ascontiguousarray(out_tab[pos_pad])
